# revision 10
# baseline (speedup 1.0000x reference)
"""MetricSelfAttention on 8 TRN2 NeuronCores — v2.

Batch-parallel SPMD (2 batches/core, no collectives), single fused pass:

  phase 1 (per 256-col slab s of W1): P-slab = x @ W1T-slab + b1 -> SBUF ph
  phase 2 (per head n, emitted 2 slabs behind production for software
           pipelining): Q = L'^T P;  S^T = (Q^T Q) masked;  OHT = P^T S^T
  phase 3: y = OH @ W2.T + b2

vs v1: no P DRAM round-trip (P stays in SBUF), host-side layouts make
every DMA contiguous per partition (fat descriptors; v1 averaged 888 B
per DMA packet and spent 52 us before the first matmul), pre_metric is
masked+scaled on host, phase-2 matmuls are interleaved between phase-1
slabs so their vector/scalar dependencies hide under the GEMM stream,
and a few warm-up matmuls run during the initial DMA fill to lift the
PE HAM clock-gate before real work arrives.

Matmul operands are bf16 (PSUM accumulation fp32); measured end-to-end
relative error vs the fp32 reference is ~5e-3.
"""

import sys

import numpy as np

try:
    import concourse.bass as bass
except ImportError:  # fresh grading dir: toolchain lives at fixed paths
    for p in ("/opt/trn_rl_repo", "/opt/pypackages"):
        if p not in sys.path:
            sys.path.insert(0, p)
    import concourse.bass as bass

import bass_rust as _bass_rust
import ml_dtypes

import concourse.mybir as mybir
from concourse.bass_utils import run_bass_kernel_spmd
from concourse.tile import TileContext
from concourse.vector_clock import ScopedClock

F32 = mybir.dt.float32
BF16 = mybir.dt.bfloat16
NP_BF16 = ml_dtypes.bfloat16
P = 128
B, W, C, N = 16, 512, 4096, 8
NCORES = 8
BL = B // NCORES  # batches per core
T = BL * W  # tokens per core
KH = C // N  # per-head dim (== W)
CSL = 256  # W1/W2 column-slab width
NSLAB = C // CSL  # 16


class PatchedTileContext(TileContext):
    """This walrus build rejects instructions carrying >1 sync wait; the
    stock exit drain carries one wait per outstanding semaphore. Spread
    them across single-wait nops instead."""

    def _drain_and_barrier(self, tick_clock, wait_clock):
        carrier = self.nc.sync.nop(nofuse=True)
        wait_clock.add_sem_waits(
            carrier.ins, ScopedClock({None: tick_clock.global_clock})
        )
        si = carrier.ins.sync_info
        waits = list(si.on_wait) if si is not None else []
        if len(waits) > 1:
            si.on_wait = waits[:1]
            for w in waits[1:]:
                extra = self.nc.sync.nop(nofuse=True)
                extra.ins.sync_info = _bass_rust.SyncInfo(on_wait=[w], on_update=[])
        self.nc.sync.drain()

        self.nc.all_engine_barrier()
        popped = self.nc._tile_sem_poison_stack.pop()
        assert popped is self._sem_poison
        self.nc.clear_and_free_semaphores(list(self.sems.allocated().values()))
        self.nc.all_engine_barrier()


def spread_sync_waits(nc):
    """Hoist all-but-one sync wait of every instruction onto single-wait
    nops inserted just before it on the same engine (queues dispatch in
    order, so semantics are preserved)."""
    k = 0
    for fn in nc.m.functions:
        for bb in fn.blocks:
            out = []
            for inst in bb.instructions:
                si = inst.sync_info
                if si is not None and len(si.on_wait) > 1:
                    waits = list(si.on_wait)
                    for w in waits[:-1]:
                        nop = mybir.InstNoOp(name=f"waitnop-{k}", ins=[], outs=[])
                        k += 1
                        nop.engine = inst.engine
                        nop.sync_info = _bass_rust.SyncInfo(on_wait=[w], on_update=[])
                        out.append(nop)
                    si.on_wait = waits[-1:]
                out.append(inst)
            bb.instructions = out


def _build():
    nc = bass.Bass()
    # Host-prepared layouts; every DMA below is contiguous per partition.
    # xh[tc][p][e*128+t'] = x_core.T[e*128+p, tc*128+t']
    xh = nc.dram_tensor("xh", [8, P, 4096], BF16, kind="ExternalInput")
    # w1h[s][p][e*256+j] = W1.T[e*128+p, s*256+j]
    w1h = nc.dram_tensor("w1h", [NSLAB, P, 32 * CSL], BF16, kind="ExternalInput")
    b1 = nc.dram_tensor("b1", [C], F32, kind="ExternalInput")
    # pmh[n][p][wc*512+v] = (pre_metric*tril*sqrt(1/sqrt(KH)))[n][wc*128+p, v]
    pmh = nc.dram_tensor("pmh", [N, P, 4 * W], BF16, kind="ExternalInput")
    # phase 3 uses 512-wide slabs (SBUF freed by then): w2h[mo][p][cc*512+j]
    w2h = nc.dram_tensor("w2h", [8, P, 32 * 512], BF16, kind="ExternalInput")
    b2 = nc.dram_tensor("b2", [C], F32, kind="ExternalInput")
    # yh[mo][tch][p][j] = y_core[tch*128+p, mo*512+j]
    yh = nc.dram_tensor("yh", [8, 8, P, 512], F32, kind="ExternalOutput")

    with PatchedTileContext(nc) as tc:
        with tc.tile_pool(name="const", bufs=1) as const, \
             tc.tile_pool(name="ln", bufs=2) as ln_pool, \
             tc.tile_pool(name="a3", bufs=1) as a_pool, \
             tc.tile_pool(name="ph", bufs=2) as ph_pool, \
             tc.tile_pool(name="mp", bufs=2) as mp_pool, \
             tc.tile_pool(name="st", bufs=2) as st_pool, \
             tc.tile_pool(name="bias", bufs=2) as b_pool, \
             tc.tile_pool(name="qps", bufs=5, space="PSUM") as qps, \
             tc.tile_pool(name="mmps", bufs=3, space="PSUM") as mmps:

            # maskf[jc][p, i] = 1.0 iff i >= jc*128 + p (tril^T; scale is
            # pre-folded into pmh on host)
            maskf = const.tile([P, 4, W], BF16)
            nc.gpsimd.memset(maskf[:, 0, :], 1.0)
            # PE warm-up during the initial DMA fill: ~4us of throwaway
            # matmuls lift the HAM clock-gate to 8/8 before real work.
            wps = qps.tile([P, W], F32, name="ps")
            for _ in range(15):
                nc.tensor.matmul(
                    wps[:], maskf[:, 0, 0:P], maskf[:, 0, :], start=True, stop=True
                )

            def ham_tick():
                # 128-col throwaway matmul: keeps the HAM activity window
                # non-idle across early DMA-starvation holes so the PE
                # clock-gate stays at 8/8 (costs ~55 ns).
                tps = qps.tile([P, W], F32, name="ps")
                nc.tensor.matmul(
                    tps[:, 0:P], maskf[:, 0, 0:P], maskf[:, 0, 0:P],
                    start=True, stop=True,
                )
            for jc in range(1, 4):
                nc.gpsimd.memset(maskf[:, jc, :], 1.0)
            for jc in range(4):
                nc.gpsimd.affine_select(
                    out=maskf[:, jc, :],
                    in_=maskf[:, jc, :],
                    compare_op=mybir.AluOpType.is_ge,
                    fill=0.0,
                    base=-jc * P,
                    pattern=[[1, W]],
                    channel_multiplier=-1,
                )

            a = a_pool.tile([P, 32, T], BF16)  # OHT: a[:, n*4+lc, t]
            ln = {}  # per-head masked-L tiles
            ph = {}  # per-head P slabs

            def load_ln(n):
                ln[n] = ln_pool.tile([P, 4 * W], BF16, name="lnt")
                nc.scalar.dma_start(ln[n][:], pmh[n])

            def q_stage(n, mp):
                # Q[v, j] = sum_w L'[w, v] P[w, j]; L'[w, v] = 0 for w < v
                lnn, phn = ln[n], ph[n]
                for b in range(BL):
                    mp[b] = mp_pool.tile([P, 4, W], BF16, name="mpt")
                    for vc in range(4):
                        ps = qps.tile([P, W], F32)
                        for wc in range(vc, 4):
                            nc.tensor.matmul(
                                ps[:],
                                lnn[:, wc * W + vc * P : wc * W + (vc + 1) * P],
                                phn[:, b * 4 + wc, :],
                                start=(wc == vc),
                                stop=(wc == 3),
                            )
                        nc.vector.tensor_copy(mp[b][:, vc, :], ps[:])

            def st_stage(n, mp, st):
                # S^T[j, i] = sum_v Q[v, j] Q[v, i], masked to i >= j: only
                # the i >= jc*128 column range is computed; maskf zeroes the
                # rest (stale psum left of the range is finite garbage).
                for b in range(BL):
                    st[b] = st_pool.tile([P, 4, W], BF16, name="stt")
                    for jc in range(4):
                        ps = qps.tile([P, W], F32)
                        for uc in range(4):
                            nc.tensor.matmul(
                                ps[:, jc * P :],
                                mp[b][:, uc, jc * P : (jc + 1) * P],
                                mp[b][:, uc, jc * P :],
                                start=(uc == 0),
                                stop=(uc == 3),
                            )
                        nc.vector.tensor_mul(st[b][:, jc, :], ps[:], maskf[:, jc, :])

            def out_stage(n, st):
                # out^T[l, i] = sum_j P[j, l] S^T[j, i]; row-chunk jc only
                # contributes to i >= jc*128 (jc=0 spans full width, start).
                phn = ph[n]
                for b in range(BL):
                    for lc in range(4):
                        ps = qps.tile([P, W], F32)
                        for jc in range(4):
                            nc.tensor.matmul(
                                ps[:, jc * P :],
                                phn[:, b * 4 + jc, lc * P : (lc + 1) * P],
                                st[b][:, jc, jc * P :],
                                start=(jc == 0),
                                stop=(jc == 3),
                            )
                        nc.scalar.copy(a[:, n * 4 + lc, b * W : (b + 1) * W], ps[:])

            def p2_head(n):
                """Scores + head output for both batches of head n.
                Pair-interleaved so each stage's vector/scalar deps are
                covered by the other pair's matmul stream."""
                mp = {}
                st = {}
                q_stage(n, mp)
                st_stage(n, mp, st)
                out_stage(n, st)

            with tc.tile_pool(name="xc", bufs=1) as xc_pool, \
                 tc.tile_pool(name="w1", bufs=2) as w1_pool:
                xc = xc_pool.tile([P, 8, 4096], BF16)
                # startup fill split across BOTH HWDGE rings in ~0.5 MB
                # pieces: each piece's ~2 us completion receipt hides under
                # the next piece's transfer, and the first matmuls begin
                # after ~1.5 MB lands per ring. SP ring serves xc 0-3,
                # ACT ring serves xc 4-7 (slab-0 groups run in matching
                # order 0,4,1,5,...).
                w1s0 = w1_pool.tile([P, 32 * CSL], BF16, name="w1t")

                def half(eng, dst, src):
                    # [2048:] first: slab-0 groups consume e 16..31 first,
                    # whose stationary operands live in the second half.
                    eng.dma_start(dst[:, 2048:], src[:, 2048:])
                    eng.dma_start(dst[:, :2048], src[:, :2048])

                half(nc.sync, xc[:, 0, :], xh[0])
                for q in range(2):
                    nc.sync.dma_start(
                        w1s0[:, q * 2048 : (q + 1) * 2048],
                        w1h[0][:, q * 2048 : (q + 1) * 2048],
                    )
                for q in range(2, 4):
                    nc.scalar.dma_start(
                        w1s0[:, q * 2048 : (q + 1) * 2048],
                        w1h[0][:, q * 2048 : (q + 1) * 2048],
                    )
                b1t_pre = {}
                for sq in range(2):
                    b1t_pre[sq] = b_pool.tile([P, CSL], F32, name="bt")
                    nc.scalar.dma_start(
                        b1t_pre[sq][:],
                        b1[sq * CSL : (sq + 1) * CSL][None, :].to_broadcast((P, CSL)),
                    )
                half(nc.scalar, xc[:, 4, :], xh[4])
                half(nc.sync, xc[:, 1, :], xh[1])
                half(nc.scalar, xc[:, 5, :], xh[5])
                half(nc.sync, xc[:, 2, :], xh[2])
                half(nc.scalar, xc[:, 6, :], xh[6])
                half(nc.sync, xc[:, 3, :], xh[3])
                half(nc.scalar, xc[:, 7, :], xh[7])
                load_ln(0)
                load_ln(1)

                for s in range(NSLAB):
                    n, h = s // 2, s % 2
                    if s == 0:
                        w1t = w1s0
                    else:
                        w1t = w1_pool.tile([P, 32 * CSL], BF16, name="w1t")
                        nc.sync.dma_start(w1t[:], w1h[s])
                    if s < 2:
                        b1t = b1t_pre[s]
                    else:
                        b1t = b_pool.tile([P, CSL], F32, name="bt")
                        nc.scalar.dma_start(
                            b1t[:],
                            b1[s * CSL : (s + 1) * CSL][None, :].to_broadcast((P, CSL)),
                        )
                    if h == 0:
                        ph[n] = ph_pool.tile([P, 8, KH], BF16, name="pht")
                    tch_order = [0, 4, 1, 5, 2, 6, 3, 7] if s == 0 else range(8)
                    # slab 0 accumulates the ACT-ring quarters (e 16..31,
                    # which land first) before the SP-ring ones so group 0
                    # starts as soon as the warm-ups finish.
                    e_order = (
                        list(range(16, 32)) + list(range(16)) if s == 0 else range(32)
                    )
                    for tch in tch_order:
                        ps = mmps.tile([P, 512], F32)
                        for k, e in enumerate(e_order):
                            nc.tensor.matmul(
                                ps[:, :CSL],
                                xc[:, tch, e * P : (e + 1) * P],
                                w1t[:, e * CSL : (e + 1) * CSL],
                                start=(k == 0),
                                stop=(k == 31),
                            )
                        nc.vector.tensor_add(
                            ph[n][:, tch, h * CSL : (h + 1) * CSL], ps[:, :CSL], b1t[:]
                        )
                        if s < 2:
                            ham_tick()
                    if s >= 3 and h == 1:
                        hd = (s - 3) // 2
                        p2_head(hd)
                        if hd + 2 < N:
                            load_ln(hd + 2)

            # ---------------- phase 3: y = OH @ W2.T + b2 ----------------
            # 512-wide slabs (xc/w1 SBUF is free now). Head 7's score
            # stages interleave with phase-3 groups: each of the first two
            # psum groups accumulates cc 0..27 (heads 0-6) between head-7
            # stages and is closed with cc 28..31 once head 7's outputs
            # exist, so the PE never idles on head 7's vector/scalar deps.
            with tc.tile_pool(name="w2", bufs=2) as w2_pool, \
                 tc.tile_pool(name="yout", bufs=3) as y_pool:
                w2 = {}
                b2t = {}
                p3ps = {}

                def load_w2(mo):
                    w2[mo] = w2_pool.tile([P, 32 * 512], BF16, name="w2t")
                    nc.sync.dma_start(w2[mo][:], w2h[mo])
                    b2t[mo] = y_pool.tile([P, 512], F32, name="b2t", bufs=2)
                    nc.scalar.dma_start(
                        b2t[mo][:],
                        b2[mo * 512 : (mo + 1) * 512][None, :].to_broadcast((P, 512)),
                    )

                def p3_acc(mo, tch, cc_lo, cc_hi):
                    if (mo, tch) not in p3ps:
                        p3ps[mo, tch] = mmps.tile([P, 512], F32, name="ps")
                    ps = p3ps[mo, tch]
                    for cc in range(cc_lo, cc_hi):
                        nc.tensor.matmul(
                            ps[:],
                            a[:, cc, tch * P : (tch + 1) * P],
                            w2[mo][:, cc * 512 : (cc + 1) * 512],
                            start=(cc == 0),
                            stop=(cc == 31),
                        )

                def p3_emit(mo, tch):
                    yo = y_pool.tile([P, 512], F32, name="yo")
                    ps = p3ps.pop((mo, tch))
                    if (mo, tch) == (7, 7):
                        # final output group: halve the add+DMA so the
                        # kernel's last HBM write (and its completion
                        # receipt) is half as long.
                        for hf in range(2):
                            sl = slice(hf * 256, (hf + 1) * 256)
                            nc.vector.tensor_add(yo[:, sl], ps[:, sl], b2t[mo][:, sl])
                            nc.sync.dma_start(yh[mo, tch][:, sl], yo[:, sl])
                    else:
                        nc.vector.tensor_add(yo[:], ps[:], b2t[mo][:])
                        nc.sync.dma_start(yh[mo, tch], yo[:])

                load_w2(0)
                mp7, st7 = {}, {}
                q_stage(7, mp7)
                p3_acc(0, 0, 0, 28)
                st_stage(7, mp7, st7)
                p3_acc(0, 1, 0, 28)
                out_stage(7, st7)
                for tch in range(2):
                    p3_acc(0, tch, 28, 32)
                    p3_emit(0, tch)
                for tch in range(2, 8):
                    p3_acc(0, tch, 0, 32)
                    p3_emit(0, tch)
                for mo in range(1, 8):
                    load_w2(mo)
                    for tch in range(8):
                        p3_acc(mo, tch, 0, 32)
                        p3_emit(mo, tch)

    spread_sync_waits(nc)
    return nc


_NC_CACHE = None
_last_in_maps = None


def kernel(**inputs: np.ndarray) -> np.ndarray:
    global _NC_CACHE, _last_in_maps
    x = np.asarray(inputs["x"], dtype=np.float32)
    W1 = np.asarray(inputs["W1"], dtype=np.float32)
    b1 = np.ascontiguousarray(np.asarray(inputs["b1"], dtype=np.float32))
    pre_metric = np.asarray(inputs["pre_metric"], dtype=np.float32)
    W2 = np.asarray(inputs["W2"], dtype=np.float32)
    b2 = np.ascontiguousarray(np.asarray(inputs["b2"], dtype=np.float32))

    def wslab(Wm, sw):  # [s][p][e*sw+j] = Wm.T[e*128+p, s*sw+j]
        ns = C // sw
        wt = np.ascontiguousarray(Wm.T).astype(NP_BF16)
        return np.ascontiguousarray(
            wt.reshape(32, P, ns, sw).transpose(2, 1, 0, 3).reshape(ns, P, 32 * sw)
        )

    w1h = wslab(W1, CSL)
    w2h = wslab(W2, 512)
    # masked lower-triangular factor with sqrt(1/sqrt(KH)) folded in
    lh = (pre_metric * np.tril(np.ones((W, W), np.float32)) * KH ** -0.25).astype(
        NP_BF16
    )
    pmh = np.ascontiguousarray(
        lh.reshape(N, 4, P, W).transpose(0, 2, 1, 3).reshape(N, P, 4 * W)
    )
    xr = x.reshape(NCORES, T, C)

    in_maps = []
    for i in range(NCORES):
        xt = np.ascontiguousarray(xr[i].T).astype(NP_BF16)  # [C, T]
        xhi = np.ascontiguousarray(
            xt.reshape(32, P, 8, P).transpose(2, 1, 0, 3).reshape(8, P, 4096)
        )
        in_maps.append(
            {"xh": xhi, "w1h": w1h, "b1": b1, "pmh": pmh, "w2h": w2h, "b2": b2}
        )

    _last_in_maps = in_maps
    if _NC_CACHE is None:
        _NC_CACHE = _build()
    res = run_bass_kernel_spmd(_NC_CACHE, in_maps, list(range(NCORES)))
    out = np.empty((B, W, C), dtype=np.float32)
    for i in range(NCORES):
        yi = res.results[i]["yh"]  # [8, 8, 128, 512]
        out[i * BL : (i + 1) * BL] = (
            yi.transpose(1, 2, 0, 3).reshape(BL, W, C).astype(np.float32)
        )
    return out


if __name__ == "__main__":
    rng = np.random.default_rng(0)
    ins = {
        "x": rng.standard_normal((B, W, C), dtype=np.float32),
        "W1": (rng.standard_normal((C, C), dtype=np.float32) * 0.02),
        "b1": (rng.standard_normal((C,), dtype=np.float32) * 0.02),
        "pre_metric": (rng.standard_normal((N, W, W), dtype=np.float32) * 0.02),
        "W2": (rng.standard_normal((C, C), dtype=np.float32) * 0.02),
        "b2": (rng.standard_normal((C,), dtype=np.float32) * 0.02),
    }
    out = kernel(**ins)
    print("kernel output shape:", out.shape, out.dtype)


# revision 11
# speedup vs baseline: 1.0007x; 1.0007x over previous
"""MetricSelfAttention on 8 TRN2 NeuronCores — v2.

Batch-parallel SPMD (2 batches/core, no collectives), single fused pass:

  phase 1 (per 256-col slab s of W1): P-slab = x @ W1T-slab + b1 -> SBUF ph
  phase 2 (per head n, emitted 2 slabs behind production for software
           pipelining): Q = L'^T P;  S^T = (Q^T Q) masked;  OHT = P^T S^T
  phase 3: y = OH @ W2.T + b2

vs v1: no P DRAM round-trip (P stays in SBUF), host-side layouts make
every DMA contiguous per partition (fat descriptors; v1 averaged 888 B
per DMA packet and spent 52 us before the first matmul), pre_metric is
masked+scaled on host, phase-2 matmuls are interleaved between phase-1
slabs so their vector/scalar dependencies hide under the GEMM stream,
and a few warm-up matmuls run during the initial DMA fill to lift the
PE HAM clock-gate before real work arrives.

Matmul operands are bf16 (PSUM accumulation fp32); measured end-to-end
relative error vs the fp32 reference is ~5e-3.
"""

import sys

import numpy as np

try:
    import concourse.bass as bass
except ImportError:  # fresh grading dir: toolchain lives at fixed paths
    for p in ("/opt/trn_rl_repo", "/opt/pypackages"):
        if p not in sys.path:
            sys.path.insert(0, p)
    import concourse.bass as bass

import bass_rust as _bass_rust
import ml_dtypes

import concourse.mybir as mybir
from concourse.bass_utils import run_bass_kernel_spmd
from concourse.tile import TileContext
from concourse.vector_clock import ScopedClock

F32 = mybir.dt.float32
BF16 = mybir.dt.bfloat16
NP_BF16 = ml_dtypes.bfloat16
P = 128
B, W, C, N = 16, 512, 4096, 8
NCORES = 8
BL = B // NCORES  # batches per core
T = BL * W  # tokens per core
KH = C // N  # per-head dim (== W)
CSL = 256  # W1/W2 column-slab width
NSLAB = C // CSL  # 16


class PatchedTileContext(TileContext):
    """This walrus build rejects instructions carrying >1 sync wait; the
    stock exit drain carries one wait per outstanding semaphore. Spread
    them across single-wait nops instead."""

    def _drain_and_barrier(self, tick_clock, wait_clock):
        carrier = self.nc.sync.nop(nofuse=True)
        wait_clock.add_sem_waits(
            carrier.ins, ScopedClock({None: tick_clock.global_clock})
        )
        si = carrier.ins.sync_info
        waits = list(si.on_wait) if si is not None else []
        if len(waits) > 1:
            si.on_wait = waits[:1]
            for w in waits[1:]:
                extra = self.nc.sync.nop(nofuse=True)
                extra.ins.sync_info = _bass_rust.SyncInfo(on_wait=[w], on_update=[])
        self.nc.sync.drain()

        self.nc.all_engine_barrier()
        popped = self.nc._tile_sem_poison_stack.pop()
        assert popped is self._sem_poison
        self.nc.clear_and_free_semaphores(list(self.sems.allocated().values()))
        self.nc.all_engine_barrier()


def spread_sync_waits(nc):
    """Hoist all-but-one sync wait of every instruction onto single-wait
    nops inserted just before it on the same engine (queues dispatch in
    order, so semantics are preserved)."""
    k = 0
    for fn in nc.m.functions:
        for bb in fn.blocks:
            out = []
            for inst in bb.instructions:
                si = inst.sync_info
                if si is not None and len(si.on_wait) > 1:
                    waits = list(si.on_wait)
                    for w in waits[:-1]:
                        nop = mybir.InstNoOp(name=f"waitnop-{k}", ins=[], outs=[])
                        k += 1
                        nop.engine = inst.engine
                        nop.sync_info = _bass_rust.SyncInfo(on_wait=[w], on_update=[])
                        out.append(nop)
                    si.on_wait = waits[-1:]
                out.append(inst)
            bb.instructions = out


def _build():
    nc = bass.Bass()
    # Host-prepared layouts; every DMA below is contiguous per partition.
    # xh[tc][p][e*128+t'] = x_core.T[e*128+p, tc*128+t']
    xh = nc.dram_tensor("xh", [8, P, 4096], BF16, kind="ExternalInput")
    # w1h[s][p][e*256+j] = W1.T[e*128+p, s*256+j]
    w1h = nc.dram_tensor("w1h", [NSLAB, P, 32 * CSL], BF16, kind="ExternalInput")
    b1 = nc.dram_tensor("b1", [C], F32, kind="ExternalInput")
    # pmh[n][p][wc*512+v] = (pre_metric*tril*sqrt(1/sqrt(KH)))[n][wc*128+p, v]
    pmh = nc.dram_tensor("pmh", [N, P, 4 * W], BF16, kind="ExternalInput")
    # phase 3 uses 512-wide slabs (SBUF freed by then): w2h[mo][p][cc*512+j]
    w2h = nc.dram_tensor("w2h", [8, P, 32 * 512], BF16, kind="ExternalInput")
    b2 = nc.dram_tensor("b2", [C], F32, kind="ExternalInput")
    # yh[mo][tch][p][j] = y_core[tch*128+p, mo*512+j]
    yh = nc.dram_tensor("yh", [8, 8, P, 512], F32, kind="ExternalOutput")

    with PatchedTileContext(nc) as tc:
        with tc.tile_pool(name="const", bufs=1) as const, \
             tc.tile_pool(name="ln", bufs=2) as ln_pool, \
             tc.tile_pool(name="a3", bufs=1) as a_pool, \
             tc.tile_pool(name="ph", bufs=2) as ph_pool, \
             tc.tile_pool(name="mp", bufs=2) as mp_pool, \
             tc.tile_pool(name="st", bufs=2) as st_pool, \
             tc.tile_pool(name="bias", bufs=2) as b_pool, \
             tc.tile_pool(name="qps", bufs=5, space="PSUM") as qps, \
             tc.tile_pool(name="mmps", bufs=3, space="PSUM") as mmps:

            # maskf[jc][p, i] = 1.0 iff i >= jc*128 + p (tril^T; scale is
            # pre-folded into pmh on host)
            maskf = const.tile([P, 4, W], BF16)
            nc.gpsimd.memset(maskf[:, 0, :], 1.0)
            # PE warm-up during the initial DMA fill: ~4us of throwaway
            # matmuls lift the HAM clock-gate to 8/8 before real work.
            wps = qps.tile([P, W], F32, name="ps")
            for _ in range(29):
                nc.tensor.matmul(
                    wps[:], maskf[:, 0, 0:P], maskf[:, 0, :], start=True, stop=True
                )

            def ham_tick():
                # 128-col throwaway matmul: keeps the HAM activity window
                # non-idle across early DMA-starvation holes so the PE
                # clock-gate stays at 8/8 (costs ~55 ns).
                tps = qps.tile([P, W], F32, name="ps")
                nc.tensor.matmul(
                    tps[:, 0:P], maskf[:, 0, 0:P], maskf[:, 0, 0:P],
                    start=True, stop=True,
                )
            for jc in range(1, 4):
                nc.gpsimd.memset(maskf[:, jc, :], 1.0)
            for jc in range(4):
                nc.gpsimd.affine_select(
                    out=maskf[:, jc, :],
                    in_=maskf[:, jc, :],
                    compare_op=mybir.AluOpType.is_ge,
                    fill=0.0,
                    base=-jc * P,
                    pattern=[[1, W]],
                    channel_multiplier=-1,
                )

            a = a_pool.tile([P, 32, T], BF16)  # OHT: a[:, n*4+lc, t]
            ln = {}  # per-head masked-L tiles
            ph = {}  # per-head P slabs

            def load_ln(n):
                ln[n] = ln_pool.tile([P, 4 * W], BF16, name="lnt")
                nc.scalar.dma_start(ln[n][:], pmh[n])

            def q_stage(n, mp):
                # Q[v, j] = sum_w L'[w, v] P[w, j]; L'[w, v] = 0 for w < v
                lnn, phn = ln[n], ph[n]
                for b in range(BL):
                    mp[b] = mp_pool.tile([P, 4, W], BF16, name="mpt")
                    for vc in range(4):
                        ps = qps.tile([P, W], F32)
                        for wc in range(vc, 4):
                            nc.tensor.matmul(
                                ps[:],
                                lnn[:, wc * W + vc * P : wc * W + (vc + 1) * P],
                                phn[:, b * 4 + wc, :],
                                start=(wc == vc),
                                stop=(wc == 3),
                            )
                        nc.vector.tensor_copy(mp[b][:, vc, :], ps[:])

            def st_stage(n, mp, st):
                # S^T[j, i] = sum_v Q[v, j] Q[v, i], masked to i >= j: only
                # the i >= jc*128 column range is computed; maskf zeroes the
                # rest (stale psum left of the range is finite garbage).
                for b in range(BL):
                    st[b] = st_pool.tile([P, 4, W], BF16, name="stt")
                    for jc in range(4):
                        ps = qps.tile([P, W], F32)
                        for uc in range(4):
                            nc.tensor.matmul(
                                ps[:, jc * P :],
                                mp[b][:, uc, jc * P : (jc + 1) * P],
                                mp[b][:, uc, jc * P :],
                                start=(uc == 0),
                                stop=(uc == 3),
                            )
                        nc.vector.tensor_mul(st[b][:, jc, :], ps[:], maskf[:, jc, :])

            def out_stage(n, st):
                # out^T[l, i] = sum_j P[j, l] S^T[j, i]; row-chunk jc only
                # contributes to i >= jc*128 (jc=0 spans full width, start).
                phn = ph[n]
                for b in range(BL):
                    for lc in range(4):
                        ps = qps.tile([P, W], F32)
                        for jc in range(4):
                            nc.tensor.matmul(
                                ps[:, jc * P :],
                                phn[:, b * 4 + jc, lc * P : (lc + 1) * P],
                                st[b][:, jc, jc * P :],
                                start=(jc == 0),
                                stop=(jc == 3),
                            )
                        nc.scalar.copy(a[:, n * 4 + lc, b * W : (b + 1) * W], ps[:])

            def p2_head(n):
                """Scores + head output for both batches of head n.
                Pair-interleaved so each stage's vector/scalar deps are
                covered by the other pair's matmul stream."""
                mp = {}
                st = {}
                q_stage(n, mp)
                st_stage(n, mp, st)
                out_stage(n, st)

            with tc.tile_pool(name="xc", bufs=1) as xc_pool, \
                 tc.tile_pool(name="w1", bufs=2) as w1_pool:
                xc = xc_pool.tile([P, 8, 4096], BF16)
                # startup fill split across BOTH HWDGE rings in ~0.5 MB
                # pieces: each piece's ~2 us completion receipt hides under
                # the next piece's transfer, and the first matmuls begin
                # after ~1.5 MB lands per ring. SP ring serves xc 0-3,
                # ACT ring serves xc 4-7 (slab-0 groups run in matching
                # order 0,4,1,5,...).
                w1s0 = w1_pool.tile([P, 32 * CSL], BF16, name="w1t")

                def half(eng, dst, src):
                    # [2048:] first: slab-0 groups consume e 16..31 first,
                    # whose stationary operands live in the second half.
                    eng.dma_start(dst[:, 2048:], src[:, 2048:])
                    eng.dma_start(dst[:, :2048], src[:, :2048])

                half(nc.sync, xc[:, 0, :], xh[0])
                for q in range(2):
                    nc.sync.dma_start(
                        w1s0[:, q * 2048 : (q + 1) * 2048],
                        w1h[0][:, q * 2048 : (q + 1) * 2048],
                    )
                for q in range(2, 4):
                    nc.scalar.dma_start(
                        w1s0[:, q * 2048 : (q + 1) * 2048],
                        w1h[0][:, q * 2048 : (q + 1) * 2048],
                    )
                b1t_pre = {}
                for sq in range(2):
                    b1t_pre[sq] = b_pool.tile([P, CSL], F32, name="bt")
                    nc.scalar.dma_start(
                        b1t_pre[sq][:],
                        b1[sq * CSL : (sq + 1) * CSL][None, :].to_broadcast((P, CSL)),
                    )
                half(nc.scalar, xc[:, 4, :], xh[4])
                half(nc.sync, xc[:, 1, :], xh[1])
                half(nc.scalar, xc[:, 5, :], xh[5])
                half(nc.sync, xc[:, 2, :], xh[2])
                half(nc.scalar, xc[:, 6, :], xh[6])
                half(nc.sync, xc[:, 3, :], xh[3])
                half(nc.scalar, xc[:, 7, :], xh[7])
                load_ln(0)
                load_ln(1)

                for s in range(NSLAB):
                    n, h = s // 2, s % 2
                    if s == 0:
                        w1t = w1s0
                    else:
                        w1t = w1_pool.tile([P, 32 * CSL], BF16, name="w1t")
                        nc.sync.dma_start(w1t[:], w1h[s])
                    if s < 2:
                        b1t = b1t_pre[s]
                    else:
                        b1t = b_pool.tile([P, CSL], F32, name="bt")
                        nc.scalar.dma_start(
                            b1t[:],
                            b1[s * CSL : (s + 1) * CSL][None, :].to_broadcast((P, CSL)),
                        )
                    if h == 0:
                        ph[n] = ph_pool.tile([P, 8, KH], BF16, name="pht")
                    tch_order = [0, 4, 1, 5, 2, 6, 3, 7] if s == 0 else range(8)
                    # slab 0 accumulates the ACT-ring quarters (e 16..31,
                    # which land first) before the SP-ring ones so group 0
                    # starts as soon as the warm-ups finish.
                    e_order = (
                        list(range(16, 32)) + list(range(16)) if s == 0 else range(32)
                    )
                    for tch in tch_order:
                        ps = mmps.tile([P, 512], F32)
                        for k, e in enumerate(e_order):
                            nc.tensor.matmul(
                                ps[:, :CSL],
                                xc[:, tch, e * P : (e + 1) * P],
                                w1t[:, e * CSL : (e + 1) * CSL],
                                start=(k == 0),
                                stop=(k == 31),
                            )
                        nc.vector.tensor_add(
                            ph[n][:, tch, h * CSL : (h + 1) * CSL], ps[:, :CSL], b1t[:]
                        )
                        if s < 2:
                            ham_tick()
                    if s >= 3 and h == 1:
                        hd = (s - 3) // 2
                        p2_head(hd)
                        if hd + 2 < N:
                            load_ln(hd + 2)

            # ---------------- phase 3: y = OH @ W2.T + b2 ----------------
            # 512-wide slabs (xc/w1 SBUF is free now). Head 7's score
            # stages interleave with phase-3 groups: each of the first two
            # psum groups accumulates cc 0..27 (heads 0-6) between head-7
            # stages and is closed with cc 28..31 once head 7's outputs
            # exist, so the PE never idles on head 7's vector/scalar deps.
            with tc.tile_pool(name="w2", bufs=2) as w2_pool, \
                 tc.tile_pool(name="yout", bufs=3) as y_pool:
                w2 = {}
                b2t = {}
                p3ps = {}

                def load_w2(mo):
                    w2[mo] = w2_pool.tile([P, 32 * 512], BF16, name="w2t")
                    nc.sync.dma_start(w2[mo][:], w2h[mo])
                    b2t[mo] = y_pool.tile([P, 512], F32, name="b2t", bufs=2)
                    nc.scalar.dma_start(
                        b2t[mo][:],
                        b2[mo * 512 : (mo + 1) * 512][None, :].to_broadcast((P, 512)),
                    )

                def p3_acc(mo, tch, cc_lo, cc_hi):
                    if (mo, tch) not in p3ps:
                        p3ps[mo, tch] = mmps.tile([P, 512], F32, name="ps")
                    ps = p3ps[mo, tch]
                    for cc in range(cc_lo, cc_hi):
                        nc.tensor.matmul(
                            ps[:],
                            a[:, cc, tch * P : (tch + 1) * P],
                            w2[mo][:, cc * 512 : (cc + 1) * 512],
                            start=(cc == 0),
                            stop=(cc == 31),
                        )

                def p3_emit(mo, tch):
                    yo = y_pool.tile([P, 512], F32, name="yo")
                    ps = p3ps.pop((mo, tch))
                    if (mo, tch) == (7, 7):
                        # final output group: halve the add+DMA so the
                        # kernel's last HBM write (and its completion
                        # receipt) is half as long.
                        for hf in range(2):
                            sl = slice(hf * 256, (hf + 1) * 256)
                            nc.vector.tensor_add(yo[:, sl], ps[:, sl], b2t[mo][:, sl])
                            nc.sync.dma_start(yh[mo, tch][:, sl], yo[:, sl])
                    else:
                        nc.vector.tensor_add(yo[:], ps[:], b2t[mo][:])
                        nc.sync.dma_start(yh[mo, tch], yo[:])

                load_w2(0)
                mp7, st7 = {}, {}
                q_stage(7, mp7)
                p3_acc(0, 0, 0, 28)
                st_stage(7, mp7, st7)
                p3_acc(0, 1, 0, 28)
                out_stage(7, st7)
                for tch in range(2):
                    p3_acc(0, tch, 28, 32)
                    p3_emit(0, tch)
                for tch in range(2, 8):
                    p3_acc(0, tch, 0, 32)
                    p3_emit(0, tch)
                for mo in range(1, 8):
                    load_w2(mo)
                    for tch in range(8):
                        p3_acc(mo, tch, 0, 32)
                        p3_emit(mo, tch)

    spread_sync_waits(nc)
    return nc


_NC_CACHE = None
_last_in_maps = None


def kernel(**inputs: np.ndarray) -> np.ndarray:
    global _NC_CACHE, _last_in_maps
    x = np.asarray(inputs["x"], dtype=np.float32)
    W1 = np.asarray(inputs["W1"], dtype=np.float32)
    b1 = np.ascontiguousarray(np.asarray(inputs["b1"], dtype=np.float32))
    pre_metric = np.asarray(inputs["pre_metric"], dtype=np.float32)
    W2 = np.asarray(inputs["W2"], dtype=np.float32)
    b2 = np.ascontiguousarray(np.asarray(inputs["b2"], dtype=np.float32))

    def wslab(Wm, sw):  # [s][p][e*sw+j] = Wm.T[e*128+p, s*sw+j]
        ns = C // sw
        wt = np.ascontiguousarray(Wm.T).astype(NP_BF16)
        return np.ascontiguousarray(
            wt.reshape(32, P, ns, sw).transpose(2, 1, 0, 3).reshape(ns, P, 32 * sw)
        )

    w1h = wslab(W1, CSL)
    w2h = wslab(W2, 512)
    # masked lower-triangular factor with sqrt(1/sqrt(KH)) folded in
    lh = (pre_metric * np.tril(np.ones((W, W), np.float32)) * KH ** -0.25).astype(
        NP_BF16
    )
    pmh = np.ascontiguousarray(
        lh.reshape(N, 4, P, W).transpose(0, 2, 1, 3).reshape(N, P, 4 * W)
    )
    xr = x.reshape(NCORES, T, C)

    in_maps = []
    for i in range(NCORES):
        xt = np.ascontiguousarray(xr[i].T).astype(NP_BF16)  # [C, T]
        xhi = np.ascontiguousarray(
            xt.reshape(32, P, 8, P).transpose(2, 1, 0, 3).reshape(8, P, 4096)
        )
        in_maps.append(
            {"xh": xhi, "w1h": w1h, "b1": b1, "pmh": pmh, "w2h": w2h, "b2": b2}
        )

    _last_in_maps = in_maps
    if _NC_CACHE is None:
        _NC_CACHE = _build()
    res = run_bass_kernel_spmd(_NC_CACHE, in_maps, list(range(NCORES)))
    out = np.empty((B, W, C), dtype=np.float32)
    for i in range(NCORES):
        yi = res.results[i]["yh"]  # [8, 8, 128, 512]
        out[i * BL : (i + 1) * BL] = (
            yi.transpose(1, 2, 0, 3).reshape(BL, W, C).astype(np.float32)
        )
    return out


if __name__ == "__main__":
    rng = np.random.default_rng(0)
    ins = {
        "x": rng.standard_normal((B, W, C), dtype=np.float32),
        "W1": (rng.standard_normal((C, C), dtype=np.float32) * 0.02),
        "b1": (rng.standard_normal((C,), dtype=np.float32) * 0.02),
        "pre_metric": (rng.standard_normal((N, W, W), dtype=np.float32) * 0.02),
        "W2": (rng.standard_normal((C, C), dtype=np.float32) * 0.02),
        "b2": (rng.standard_normal((C,), dtype=np.float32) * 0.02),
    }
    out = kernel(**ins)
    print("kernel output shape:", out.shape, out.dtype)
